# revision 2
# baseline (speedup 1.0000x reference)
"""Trainium2 Bass kernel for a 2-layer GCN (nn_MetaEncoder).

Reference computation (per layer, A-hat = normalized adjacency w/ self loops):
    h   = x @ W.T
    agg = A_hat @ h + b          (A_hat row i: norm over incoming edges + self)
    layer1: r = relu(agg1);  layer2: out = agg2

Distribution strategy (8 NeuronCores, SPMD single program):
  - Nodes (rows) sharded by destination: core k owns dst rows
    [k*N/8, (k+1)*N/8).  Edges partitioned by dst, sorted by dst.
  - Layer 1 uses linearity: agg1 = (A_hat @ x) @ W1.T, so each core gathers
    x rows (x is replicated in every core's DRAM as a kernel input) and
    aggregates FIRST, then does the small dense matmuls for its shard.
    -> no collective needed for layer 1.
  - Layer 2: each core computes h2_k = r_k @ W2.T for its shard, an
    AllGather assembles h2_full [N, COUT] in every core's DRAM, then each
    core gathers h2 rows for its incoming edges and aggregates.
  - Aggregation runs on the tensor engine: edges (sorted by dst) are
    processed in tiles of 128; a per-tile "scaled one-hot" matrix
    S[e, d] = norm_e * (dst_local_e == d) is built on the vector engine
    (iota + compare + scale), and   psum[dst, ch] += S.T @ gathered_rows
    accumulates a whole 128-dst block in one PSUM bank.
  - Row gathers use the SWDGE dma_gather instruction (int16 indices, so the
    node table is split in two halves < 32768 rows each).

The whole 2-layer network is ONE NEFF launch per core (collective inside).
"""

import math
import os
import sys

import numpy as np

for _p in ("/opt/trn_rl_repo",):
    if _p not in sys.path and os.path.isdir(_p):
        sys.path.append(_p)

import concourse.bacc as bacc
import concourse.bass as bass
import concourse.tile as tile
from concourse import mybir

P = 128
NCORES = 8
F32 = mybir.dt.float32
F32R = mybir.dt.float32r
I16 = mybir.dt.int16


# ----------------------------------------------------------------------------
# Host-side preprocessing: shard edges by destination, sort, pad, build the
# per-core index / one-hot metadata arrays.
# ----------------------------------------------------------------------------
class Plan:
    pass


def preprocess(x, edge_index, w1, b1, w2, b2, t_ch1=8, t_ch2=16):
    N, CIN = x.shape
    CH = w1.shape[0]  # hidden width (2*COUT)
    COUT = w2.shape[0]
    E = edge_index.shape[1]
    assert N % NCORES == 0
    NLOC = N // NCORES
    NB = math.ceil(NLOC / P)
    # split point for int16 gather indices (both halves must be < 32768)
    SPLIT = math.ceil(N / 2 / P) * P
    assert SPLIT < 32768 and (N - SPLIT) < 32768

    src = np.asarray(edge_index[0], dtype=np.int64)
    dst = np.asarray(edge_index[1], dtype=np.int64)
    deg = (np.bincount(dst, minlength=N) + 1.0).astype(np.float32)
    dinv = (1.0 / np.sqrt(deg)).astype(np.float32)
    norm = (dinv[src] * dinv[dst]).astype(np.float32)

    # append self edges (weight dinv^2) so aggregation handles self loops
    allsrc = np.concatenate([src, np.arange(N, dtype=np.int64)])
    alldst = np.concatenate([dst, np.arange(N, dtype=np.int64)])
    allw = np.concatenate([norm, dinv * dinv]).astype(np.float32)

    order = np.argsort(alldst, kind="stable")
    allsrc, alldst, allw = allsrc[order], alldst[order], allw[order]

    core_lo = np.searchsorted(alldst, np.arange(NCORES) * NLOC)
    core_hi = np.searchsorted(alldst, (np.arange(NCORES) + 1) * NLOC)

    # per (core, block): lo-half and hi-half edge runs
    runs = []  # [core][block] -> (lo_src, lo_dstb, lo_w, hi_src, hi_dstb, hi_w)
    nlo = np.zeros((NCORES, NB), dtype=np.int64)
    nhi = np.zeros((NCORES, NB), dtype=np.int64)
    for k in range(NCORES):
        s, e = core_lo[k], core_hi[k]
        csrc, cdst, cw = allsrc[s:e], alldst[s:e] - k * NLOC, allw[s:e]
        bbounds = np.searchsorted(cdst, np.arange(NB + 1) * P)
        per_block = []
        for b in range(NB):
            s0, e0 = bbounds[b], bbounds[b + 1]
            bs, bd, bw = csrc[s0:e0], cdst[s0:e0] - b * P, cw[s0:e0]
            lo = bs < SPLIT
            per_block.append(
                (bs[lo], bd[lo], bw[lo], bs[~lo] - SPLIT, bd[~lo], bw[~lo])
            )
            nlo[k, b] = int(lo.sum())
            nhi[k, b] = int((~lo).sum())
        runs.append(per_block)

    # uniform tile counts across cores (SPMD: one program for all cores)
    Tlo = np.maximum(np.ceil(nlo / P).max(axis=0), 0).astype(np.int64)
    Thi = np.maximum(np.ceil(nhi / P).max(axis=0), 0).astype(np.int64)
    # every block needs >= 1 tile so the PSUM accumulation group is non-empty
    for b in range(NB):
        if Tlo[b] + Thi[b] == 0:
            Tlo[b] = 1
    T_total = int((Tlo + Thi).sum())
    L = T_total * P

    # build padded per-core streams
    idx16 = np.zeros((NCORES, L), dtype=np.int16)
    dstb = np.zeros((NCORES, L), dtype=np.float32)
    wgt = np.zeros((NCORES, L), dtype=np.float32)
    for k in range(NCORES):
        pos = 0
        for b in range(NB):
            ls, ld, lw, hs, hd, hw = runs[k][b]
            for (rs, rd, rw), T in (((ls, ld, lw), Tlo[b]), ((hs, hd, hw), Thi[b])):
                n = len(rs)
                Lr = int(T) * P
                assert n <= Lr
                idx16[k, pos : pos + n] = rs.astype(np.int16)
                dstb[k, pos : pos + n] = rd.astype(np.float32)
                wgt[k, pos : pos + n] = rw
                # padding: idx 0 (valid row), weight 0 -> contributes nothing
                pos += Lr
        assert pos == L

    # device layouts
    #   idx16: wrapped [16, L/16] (idx j at [j%16, j//16]) replicated to 128 p
    idx_dev = np.tile(
        idx16.reshape(NCORES, L // 16, 16).transpose(0, 2, 1), (1, 8, 1)
    )  # [NCORES, 128, L/16]
    #   dstb/w: [128, T_total] with edge t*128+p at [p, t]
    dstb_dev = dstb.reshape(NCORES, T_total, P).transpose(0, 2, 1).copy()
    wgt_dev = wgt.reshape(NCORES, T_total, P).transpose(0, 2, 1).copy()

    IC = CIN // P
    OC = CH // P
    w1t = np.ascontiguousarray(
        np.asarray(w1, np.float32).T.reshape(IC, P, CH).transpose(1, 0, 2)
    )  # [128, IC, CH]
    w2t = np.ascontiguousarray(
        np.asarray(w2, np.float32).T.reshape(OC, P, COUT).transpose(1, 0, 2)
    )  # [128, OC, COUT]
    b1c = np.ascontiguousarray(np.asarray(b1, np.float32).reshape(OC, P).T)  # [128,OC]
    b2r = np.ascontiguousarray(
        np.broadcast_to(np.asarray(b2, np.float32), (P, COUT))
    )  # [128, COUT]
    # consts: [iota | identity]
    iota = np.broadcast_to(np.arange(P, dtype=np.float32), (P, P))
    ident = np.eye(P, dtype=np.float32)
    consts = np.ascontiguousarray(np.concatenate([iota, ident], axis=1))  # [128,256]

    pl = Plan()
    pl.N, pl.CIN, pl.CH, pl.COUT, pl.E = N, CIN, CH, COUT, E
    pl.NLOC, pl.NB, pl.SPLIT = NLOC, NB, SPLIT
    pl.IC, pl.OC = IC, OC
    pl.Tlo, pl.Thi, pl.T_total, pl.L = Tlo, Thi, T_total, L
    pl.t_ch1, pl.t_ch2 = t_ch1, t_ch2
    pl.x = np.ascontiguousarray(np.asarray(x, np.float32))
    pl.idx_dev, pl.dstb_dev, pl.wgt_dev = idx_dev, dstb_dev, wgt_dev
    pl.w1t, pl.w2t, pl.b1c, pl.b2r, pl.consts = w1t, w2t, b1c, b2r, consts
    return pl


# ----------------------------------------------------------------------------
# Bass program (SPMD, same for all 8 cores)
# ----------------------------------------------------------------------------
def build_program(pl):
    nc = bacc.Bacc(
        "TRN2",
        target_bir_lowering=False,
        debug=False,
        enable_asserts=True,
        num_devices=NCORES,
    )
    N, CIN, CH, COUT = pl.N, pl.CIN, pl.CH, pl.COUT
    NLOC, NB, SPLIT = pl.NLOC, pl.NB, pl.SPLIT
    IC, OC = pl.IC, pl.OC
    T_total, L = pl.T_total, pl.L
    NI16 = L // 16

    x_t = nc.dram_tensor("x", [N, CIN], F32, kind="ExternalInput")
    idx_t = nc.dram_tensor("idx16", [P, NI16], I16, kind="ExternalInput")
    dstb_t = nc.dram_tensor("dstb", [P, T_total], F32, kind="ExternalInput")
    wgt_t = nc.dram_tensor("wgt", [P, T_total], F32, kind="ExternalInput")
    w1t_t = nc.dram_tensor("w1t", [P, IC * CH], F32, kind="ExternalInput")
    w2t_t = nc.dram_tensor("w2t", [P, OC * COUT], F32, kind="ExternalInput")
    b1c_t = nc.dram_tensor("b1c", [P, OC], F32, kind="ExternalInput")
    b2r_t = nc.dram_tensor("b2r", [P, COUT], F32, kind="ExternalInput")
    consts_t = nc.dram_tensor("consts", [P, 2 * P], F32, kind="ExternalInput")
    h2loc_t = nc.dram_tensor("h2loc", [NLOC, COUT], F32)
    h2full_t = nc.dram_tensor("h2full", [N, COUT], F32, addr_space="Shared")
    out_t = nc.dram_tensor("out", [NLOC, COUT], F32, kind="ExternalOutput")

    r = lambda ap: ap  # fp32 matmuls (fp32r needs producer-side rounding)

    with tile.TileContext(nc) as tc:
        with tc.tile_pool(name="const", bufs=1) as cp:
            consts_sb = cp.tile([P, 2 * P], F32)
            nc.sync.dma_start(consts_sb[:], consts_t[:])
            iota_ap = consts_sb[:, 0:P]
            ident_ap = consts_sb[:, P : 2 * P]
            idx_sb = cp.tile([P, NI16], I16)
            nc.sync.dma_start(idx_sb[:], idx_t[:])
            dstb_sb = cp.tile([P, T_total], F32)
            nc.sync.dma_start(dstb_sb[:], dstb_t[:])
            wgt_sb = cp.tile([P, T_total], F32)
            nc.sync.dma_start(wgt_sb[:], wgt_t[:])
            w1t_sb = cp.tile([P, IC * CH], F32)
            nc.sync.dma_start(w1t_sb[:], w1t_t[:])
            w3 = w1t_sb[:].rearrange("p (i c) -> p i c", c=CH)
            w2t_sb = cp.tile([P, OC * COUT], F32)
            nc.sync.dma_start(w2t_sb[:], w2t_t[:])
            v3 = w2t_sb[:].rearrange("p (o c) -> p o c", c=COUT)
            b1_sb = cp.tile([P, OC], F32)
            nc.sync.dma_start(b1_sb[:], b1c_t[:])
            b2_sb = cp.tile([P, COUT], F32)
            nc.sync.dma_start(b2_sb[:], b2r_t[:])

            # ---------------- phase A: layer1 aggregate + dense + h2 ------
            with (
                tc.tile_pool(name="xg", bufs=3) as xgp,
                tc.tile_pool(name="oh", bufs=4) as ohp,
                tc.tile_pool(name="aggps", bufs=2, space="PSUM") as aggp,
                tc.tile_pool(name="trps", bufs=2, space="PSUM") as trp,
                tc.tile_pool(name="aggs", bufs=2) as aggsp,
                tc.tile_pool(name="aggt", bufs=2) as aggtp,
                tc.tile_pool(name="h1ps", bufs=2, space="PSUM") as h1p,
                tc.tile_pool(name="rt", bufs=2) as rtp,
                tc.tile_pool(name="h2ps", bufs=2, space="PSUM") as h2p,
                tc.tile_pool(name="h2sb", bufs=2) as h2sbp,
            ):
                tcur = 0  # global edge-tile cursor
                NSB = math.ceil(NB / 2)
                for s in range(NSB):
                    blocks = [b for b in (2 * s, 2 * s + 1) if b < NB]
                    nn = sum(min(P, NLOC - b * P) for b in blocks)
                    aggT = aggtp.tile([P, IC * 2 * P], F32)
                    a3 = aggT[:].rearrange("p (i n) -> p i n", n=2 * P)
                    for bh, b in enumerate(blocks):
                        nb_rows = min(P, NLOC - b * P)
                        T_b = int(pl.Tlo[b] + pl.Thi[b])
                        agg_ps = aggp.tile([P, CIN], F32, space="PSUM")
                        tloc = 0
                        for half, T_run in ((0, int(pl.Tlo[b])), (1, int(pl.Thi[b]))):
                            if T_run == 0:
                                continue
                            table = (
                                x_t[0:SPLIT, :] if half == 0 else x_t[SPLIT:N, :]
                            )
                            for c0 in range(0, T_run, pl.t_ch1):
                                n_t = min(pl.t_ch1, T_run - c0)
                                xg = xgp.tile([P, pl.t_ch1 * CIN], F32)
                                x3 = xg[:].rearrange("p (t c) -> p t c", c=CIN)
                                e0 = (tcur + tloc) * P
                                nc.gpsimd.dma_gather(
                                    x3[:, 0:n_t, :],
                                    table,
                                    idx_sb[:, e0 // 16 : (e0 + n_t * P) // 16],
                                    n_t * P,
                                    n_t * P,
                                    CIN,
                                )
                                for ti in range(n_t):
                                    tg = tcur + tloc
                                    oh = ohp.tile([P, P], F32)
                                    nc.vector.tensor_scalar(
                                        oh[:],
                                        iota_ap,
                                        dstb_sb[:, tg : tg + 1],
                                        wgt_sb[:, tg : tg + 1],
                                        mybir.AluOpType.is_equal,
                                        mybir.AluOpType.mult,
                                    )
                                    nc.tensor.matmul(
                                        agg_ps[:],
                                        r(oh[:]),
                                        r(x3[:, ti, :]),
                                        start=(tloc == 0),
                                        stop=(tloc == T_b - 1),
                                    )
                                    tloc += 1
                        tcur += T_b
                        # transpose agg [dst, ch] -> aggT [ch, dst]
                        aggS = aggsp.tile([P, CIN], F32)
                        nc.vector.tensor_copy(aggS[:], agg_ps[:])
                        for ic in range(IC):
                            tr_ps = trp.tile([P, P], F32, space="PSUM")
                            nc.tensor.transpose(
                                tr_ps[:, 0:nb_rows],
                                aggS[0:nb_rows, ic * P : (ic + 1) * P],
                                ident_ap[0:nb_rows, 0:nb_rows],
                            )
                            nc.vector.tensor_copy(
                                a3[:, ic, bh * P : bh * P + nb_rows],
                                tr_ps[:, 0:nb_rows],
                            )
                    # dense: h1T = W1 @ aggT (+b1, relu) ; h2 = rT.T @ W2T
                    rT = rtp.tile([P, OC * 2 * P], F32)
                    r3 = rT[:].rearrange("p (o n) -> p o n", n=2 * P)
                    for oc in range(OC):
                        h1_ps = h1p.tile([P, 2 * P], F32, space="PSUM")
                        for ic in range(IC):
                            nc.tensor.matmul(
                                h1_ps[:, 0:nn],
                                r(w3[:, ic, oc * P : (oc + 1) * P]),
                                r(a3[:, ic, 0:nn]),
                                start=(ic == 0),
                                stop=(ic == IC - 1),
                            )
                        nc.scalar.activation(
                            r3[:, oc, 0:nn],
                            h1_ps[:, 0:nn],
                            mybir.ActivationFunctionType.Relu,
                            bias=b1_sb[:, oc : oc + 1],
                            scale=1.0,
                        )
                    for nh, b in enumerate(blocks):
                        nrows = min(P, NLOC - b * P)
                        h2_ps = h2p.tile([P, COUT], F32, space="PSUM")
                        for oc in range(OC):
                            nc.tensor.matmul(
                                h2_ps[0:nrows, :],
                                r(r3[:, oc, nh * P : nh * P + nrows]),
                                r(v3[:, oc, :]),
                                start=(oc == 0),
                                stop=(oc == OC - 1),
                            )
                        h2sb = h2sbp.tile([P, COUT], F32)
                        nc.vector.tensor_copy(h2sb[0:nrows, :], h2_ps[0:nrows, :])
                        nc.sync.dma_start(
                            h2loc_t[b * P : b * P + nrows, :], h2sb[0:nrows, :]
                        )
                assert tcur == T_total

            # ---------------- AllGather h2 shards -------------------------
            nc.gpsimd.collective_compute(
                "AllGather",
                mybir.AluOpType.bypass,
                replica_groups=[list(range(NCORES))],
                ins=[h2loc_t[:]],
                outs=[h2full_t[:]],
            )

            # ---------------- phase C: layer2 aggregate + bias ------------
            with (
                tc.tile_pool(name="h2g", bufs=3) as h2gp,
                tc.tile_pool(name="oh2", bufs=4) as ohp2,
                tc.tile_pool(name="outps", bufs=4, space="PSUM") as outp,
                tc.tile_pool(name="outsb", bufs=2) as outsbp,
            ):
                tcur = 0
                for b in range(NB):
                    nb_rows = min(P, NLOC - b * P)
                    T_b = int(pl.Tlo[b] + pl.Thi[b])
                    out_ps = outp.tile([P, COUT], F32, space="PSUM")
                    tloc = 0
                    for half, T_run in ((0, int(pl.Tlo[b])), (1, int(pl.Thi[b]))):
                        if T_run == 0:
                            continue
                        table = (
                            h2full_t[0:SPLIT, :] if half == 0 else h2full_t[SPLIT:N, :]
                        )
                        for c0 in range(0, T_run, pl.t_ch2):
                            n_t = min(pl.t_ch2, T_run - c0)
                            hg = h2gp.tile([P, pl.t_ch2 * COUT], F32)
                            g3 = hg[:].rearrange("p (t c) -> p t c", c=COUT)
                            e0 = (tcur + tloc) * P
                            nc.gpsimd.dma_gather(
                                g3[:, 0:n_t, :],
                                table,
                                idx_sb[:, e0 // 16 : (e0 + n_t * P) // 16],
                                n_t * P,
                                n_t * P,
                                COUT,
                            )
                            for ti in range(n_t):
                                tg = tcur + tloc
                                oh = ohp2.tile([P, P], F32)
                                nc.vector.tensor_scalar(
                                    oh[:],
                                    iota_ap,
                                    dstb_sb[:, tg : tg + 1],
                                    wgt_sb[:, tg : tg + 1],
                                    mybir.AluOpType.is_equal,
                                    mybir.AluOpType.mult,
                                )
                                nc.tensor.matmul(
                                    out_ps[:],
                                    r(oh[:]),
                                    r(g3[:, ti, :]),
                                    start=(tloc == 0),
                                    stop=(tloc == T_b - 1),
                                )
                                tloc += 1
                    tcur += T_b
                    outsb = outsbp.tile([P, COUT], F32)
                    nc.vector.tensor_tensor(
                        out=outsb[0:nb_rows, :],
                        in0=out_ps[0:nb_rows, :],
                        in1=b2_sb[0:nb_rows, :],
                        op=mybir.AluOpType.add,
                    )
                    nc.sync.dma_start(
                        out_t[b * P : b * P + nb_rows, :], outsb[0:nb_rows, :]
                    )
                assert tcur == T_total

    nc.compile()
    return nc


def make_in_maps(pl):
    maps = []
    for k in range(NCORES):
        maps.append(
            {
                "x": pl.x,
                "idx16": np.ascontiguousarray(pl.idx_dev[k]),
                "dstb": np.ascontiguousarray(pl.dstb_dev[k]),
                "wgt": np.ascontiguousarray(pl.wgt_dev[k]),
                "w1t": pl.w1t.reshape(P, -1),
                "w2t": pl.w2t.reshape(P, -1),
                "b1c": pl.b1c,
                "b2r": pl.b2r,
                "consts": pl.consts,
            }
        )
    return maps


def kernel(x, edge_index, w1, b1, w2, b2):
    from concourse.bass_utils import run_bass_kernel_spmd

    pl = preprocess(x, edge_index, w1, b1, w2, b2)
    nc = build_program(pl)
    res = run_bass_kernel_spmd(nc, make_in_maps(pl), list(range(NCORES)))
    out = np.concatenate([res.results[k]["out"] for k in range(NCORES)], axis=0)
    return out.astype(np.float32)
